# revision 29
# baseline (speedup 1.0000x reference)
"""Multi-head cross-attention Trainium2 Bass kernel, SPMD over 8 NeuronCores.

Sharding: core c handles batch b = c//2 and head group g = c%2 (8 of 16 heads).
Each core computes a partial output projection (its heads' W_o rows); the host
sums the two partials per batch element.

Device pipeline per core (all matmuls bf16 with fp32 PSUM accumulation):
  kT = (Wk^T x^T)          [512 hd, 2048 kseq]   (per-partition bias b_k)
  v  = (x Wv)              [2048 kseq, 8*65]     (65th col per head = ones)
  qT = (Wq^T y^T)          [512 hd, 1024 q]      (per-partition bias b_q)
  per (head-pair, q-tile, k-chunk):
      S^T[k, q|q'] = kT_h^T-chunk @ qT_h for both heads of the pair
        (K=64 row-tiled at partitions 0/64 -> the two matmuls run
         concurrently in the PE array; both write one 2-bank PSUM tile)
      em = exp(0.125 * S^T)  (one ACT op per pair; no row-max: |S|<=~25)
      em *= maskT            (one DVE mul per pair; the mask chunk is read
                              once via a step-0 broadcast dim — multiplicative
                              masking == additive -inf pre-exp)
      acc_h[65, q] += [v_h | 1]^T @ em_h   (row 64 = softmax denominator)
  normalize: vals_h = acc[0:64] * bcast(1/acc[64])  (PE outer-product bcast,
      reciprocal_approx_fast; deferred into the next head-pair's loop)
  out_partial = vals^T-chunks @ Wo-rows  -> [1024 q, 1024 D] fp32
The kT/qT/v projection matmuls are interleaved into the attention loop to
keep the PE dense (HAM stays un-throttled at 2.4 GHz); xT streams in column
halves so the first projection chains start during the input DMA.
b_v and b_o fold into a host-side constant row (attn rows sum to 1).
"""

import sys
from collections import deque

import numpy as np
import ml_dtypes

if "/opt/trn_rl_repo" not in sys.path:
    sys.path.insert(0, "/opt/trn_rl_repo")

BF = ml_dtypes.bfloat16

B, NKV, NQ, D, H = 4, 2048, 1024, 1024, 16
HD = D // H          # 64
NHL = 8              # heads per core (local)
P = 128
DC = D // P          # 8 contraction chunks over model dim
KC = NKV // P        # 16 key-seq chunks
QT = NQ // 512       # 2 q tiles of 512 for attention
MT = 4               # hd-dim chunks of kT/qT (512/128)

_CACHE = {}


def _build_program():
    import concourse.bass as bass
    import concourse.mybir as mybir
    import concourse.tile as tile
    from concourse import bacc

    f32 = mybir.dt.float32
    bf16 = mybir.dt.bfloat16

    nc = bacc.Bacc(
        "TRN2", target_bir_lowering=False, debug=False, num_devices=8
    )

    xT_d = nc.dram_tensor("xT", [D, NKV], bf16, kind="ExternalInput").ap()
    yT_d = nc.dram_tensor("yT", [D, NQ], bf16, kind="ExternalInput").ap()
    maskT_d = nc.dram_tensor("maskT", [NKV, NQ], bf16, kind="ExternalInput").ap()
    wk_d = nc.dram_tensor("wk", [D, 512], bf16, kind="ExternalInput").ap()
    wv_d = nc.dram_tensor("wv", [D, 512], bf16, kind="ExternalInput").ap()
    wq_d = nc.dram_tensor("wq", [D, 512], bf16, kind="ExternalInput").ap()
    wo_d = nc.dram_tensor("wo", [512, D], bf16, kind="ExternalInput").ap()
    bk_d = nc.dram_tensor("bk", [512, 1], f32, kind="ExternalInput").ap()
    bq_d = nc.dram_tensor("bq", [512, 1], f32, kind="ExternalInput").ap()
    out_d = nc.dram_tensor("out", [NQ, D], f32, kind="ExternalOutput").ap()

    Exp = mybir.ActivationFunctionType.Exp

    with tile.TileContext(nc) as tc:
        with (
            tc.tile_pool(name="persist", bufs=1) as persist,
            tc.tile_pool(name="work", bufs=3) as work,
            tc.tile_pool(name="empool", bufs=4) as empool,
            tc.tile_pool(name="pmm", bufs=2, space="PSUM") as pmm,
            tc.tile_pool(name="pacc", bufs=2, space="PSUM") as pacc,
            tc.tile_pool(name="psc", bufs=2, space="PSUM") as psc,
        ):
            def row_tile(nchunks, cols, dtype, label):
                return [
                    persist.tile(
                        [P, cols], dtype, tag=f"{label}{i}", name=f"{label}{i}"
                    )
                    for i in range(nchunks)
                ]

            def load(tiles, dram, i):
                nc.sync.dma_start(tiles[i], dram[i * P:(i + 1) * P, :])

            wk_sb = row_tile(DC, 512, bf16, "wk")
            wv_sb = row_tile(DC, 512, bf16, "wv")
            xT_sb = row_tile(DC, NKV, bf16, "xT")
            wq_sb = row_tile(DC, 512, bf16, "wq")
            yT_sb = row_tile(DC, NQ, bf16, "yT")
            bk_sb = row_tile(MT, 1, f32, "bk")
            bq_sb = row_tile(MT, 1, f32, "bq")
            maskT_sb = row_tile(KC, NQ, bf16, "mT")
            wo_sb = row_tile(MT, D, bf16, "wo")

            # DMA issue order = consumption order. xT loads in column halves
            # so the first kT chains (n=0,1) and early v chains start sooner.
            def load_xt_half(d, h):
                nc.sync.dma_start(
                    xT_sb[d][:, h * 1024:(h + 1) * 1024],
                    xT_d[d * P:(d + 1) * P, h * 1024:(h + 1) * 1024],
                )

            for d in range(2):
                load(wk_sb, wk_d, d)
                load_xt_half(d, 0)
            for m in range(MT):
                load(bk_sb, bk_d, m)
                load(bq_sb, bq_d, m)
            for d in range(2, DC):
                load(wk_sb, wk_d, d)
                load_xt_half(d, 0)
            for d in range(DC):
                load(wv_sb, wv_d, d)
            for d in range(DC):
                load_xt_half(d, 1)
            for d in range(DC):
                load(wq_sb, wq_d, d)
                load(yT_sb, yT_d, d)
            for i in range(KC):
                load(maskT_sb, maskT_d, i)
            for m in range(MT):
                load(wo_sb, wo_d, m)

            ones_sb = persist.tile([1, HD], bf16, tag="ones", name="ones")
            nc.gpsimd.memset(ones_sb, 1.0)
            # v ones-columns never overlap the v copy's columns; set them once
            # up front, off the critical path.

            kT_sb = [
                persist.tile([P, NKV], bf16, tag=f"kT{m}", name=f"kT{m}")
                for m in range(MT)
            ]
            qT_sb = [
                persist.tile([P, NQ], bf16, tag=f"qT{m}", name=f"qT{m}")
                for m in range(MT)
            ]
            v_sb = [
                persist.tile([P, NHL * 65], bf16, tag=f"v{i}", name=f"v{i}")
                for i in range(KC)
            ]
            for i in range(KC):
                nc.gpsimd.memset(
                    v_sb[i].rearrange("p (h c) -> p h c", c=65)[:, :, 64:65], 1.0
                )
            vals_sb = [
                persist.tile([P, NQ], bf16, tag=f"vals{c}", name=f"vals{c}")
                for c in range(MT)
            ]

            # ---- projection op queues (kT/qT chunk m as a list of closures,
            # one matmul each; the chain's last op appends the bias-add) ----
            def proj_ops(m, which, ns=None):
                w_sb, dst, bias, ncols = (
                    (wk_sb, kT_sb, bk_sb, NKV) if which == "k"
                    else (wq_sb, qT_sb, bq_sb, NQ)
                )
                ops = []
                hold = {}
                for n in (range(ncols // 512) if ns is None else ns):
                    for d in range(DC):
                        def op(m=m, n=n, d=d, w_sb=w_sb, dst=dst, bias=bias,
                               which=which):
                            if d == 0:
                                hold[n] = pmm.tile(
                                    [P, 512], f32, tag="mm",
                                    name=f"pj{which}{m}_{n}"
                                )
                            nc.tensor.matmul(
                                hold[n],
                                lhsT=w_sb[d][:, m * P:(m + 1) * P],
                                rhs=(xT_sb if which == "k" else yT_sb)[d][
                                    :, n * 512:(n + 1) * 512],
                                start=(d == 0),
                                stop=(d == DC - 1),
                            )
                            if d == DC - 1:
                                nc.vector.tensor_scalar_add(
                                    dst[m][:, n * 512:(n + 1) * 512],
                                    hold[n], bias[m]
                                )
                        ops.append(op)
                return ops

            def emit_v(i):
                ps_v = pmm.tile([P, 512], f32, tag="mm", name=f"ps_v{i}")
                for d in range(DC):
                    nc.tensor.matmul(
                        ps_v,
                        lhsT=xT_sb[d][:, i * P:(i + 1) * P],
                        rhs=wv_sb[d],
                        start=(d == 0),
                        stop=(d == DC - 1),
                    )
                v3 = v_sb[i].rearrange("p (h c) -> p h c", c=65)
                nc.vector.tensor_copy(
                    v3[:, :, 0:64], ps_v.rearrange("p (h c) -> p h c", c=64)
                )

            # upfront: kT/qT chunk 0 (needed by head-pair 0) and the first
            # half of v — the PE would otherwise idle in the initial
            # DMA-wait window. Emission follows DMA arrival order: kT n0/n1
            # (xT half 0), early v chains, then kT n2/n3 (xT half 1).
            for op in proj_ops(0, "k", ns=[0, 1]):
                op()
            for i in range(KC // 2):
                emit_v(i)
            for op in proj_ops(0, "k", ns=[2, 3]):
                op()
            for op in proj_ops(0, "q"):
                op()

            # per-head-pair deferred projections, drained during attention
            pending = {
                0: deque(proj_ops(1, "k") + proj_ops(1, "q")),
                1: deque(proj_ops(2, "k") + proj_ops(2, "q")),
                2: deque(proj_ops(3, "k") + proj_ops(3, "q")),
                3: deque(),
            }

            # ---- attention (projections + deferred normalize interleaved) ----
            norm_pending = deque()

            def make_norm(hp, t, a, h, ut, s_f):
                po = a * HD
                qs = slice(t * 512, (t + 1) * 512)

                def norm_op():
                    r_f = work.tile([1, 512], f32, tag="r", name=f"r{h}_{t}")
                    nc.vector.reciprocal_approx_fast(r_f, s_f)
                    r_b = work.tile([1, 512], bf16, tag="rb", name=f"rb{h}_{t}")
                    nc.vector.tensor_copy(r_b, r_f)
                    bps = pmm.tile([HD, 512], f32, tag="mm", name=f"bps{h}_{t}")
                    nc.tensor.matmul(
                        bps, lhsT=ones_sb, rhs=r_b, start=True, stop=True
                    )
                    nc.vector.tensor_mul(vals_sb[hp][po:po + HD, qs], ut, bps)
                return norm_op

            for hp in range(NHL // 2):
                h0, h1 = 2 * hp, 2 * hp + 1
                q = pending[hp]
                for t in range(QT):
                    qs = slice(t * 512, (t + 1) * 512)
                    slots_left = (QT - t) * KC
                    accs = [
                        pacc.tile([65, 512], f32, tag="acc", name=f"acc{h}_{t}")
                        for h in (h0, h1)
                    ]
                    for kc in range(KC):
                        if hp == 0 and t == 0:
                            if kc >= KC // 2:
                                emit_v(kc)  # second v half, ready before use
                        else:
                            n_emit = -(-len(q) // slots_left)  # ceil
                            for _ in range(min(n_emit, len(q))):
                                q.popleft()()
                        slots_left -= 1
                        if kc % 4 == 2 and norm_pending:
                            norm_pending.popleft()()

                        sp2 = psc.tile(
                            [P, 1024], f32, tag="sc", name=f"sp{hp}_{t}_{kc}"
                        )
                        for a in range(2):
                            po = a * HD
                            nc.tensor.matmul(
                                sp2[:, a * 512:(a + 1) * 512],
                                lhsT=kT_sb[hp][po:po + HD, kc * P:(kc + 1) * P],
                                rhs=qT_sb[hp][po:po + HD, qs],
                                start=True,
                                stop=True,
                            )
                        em2 = empool.tile(
                            [P, 1024], bf16, tag="em", name=f"em{hp}_{t}_{kc}"
                        )
                        nc.scalar.activation(em2, sp2, Exp, scale=0.125)
                        # one masked multiply for both heads: the mask chunk is
                        # read once and broadcast (step-0 dim) over the pair
                        mb = (maskT_sb[kc][:, qs]
                              .rearrange("p (o q) -> p o q", o=1)
                              .broadcast_to([P, 2, 512]))
                        em3 = em2.rearrange("p (o q) -> p o q", o=2)
                        nc.vector.tensor_mul(em3, em3, mb)
                        for a, h in enumerate((h0, h1)):
                            nc.tensor.matmul(
                                accs[a],
                                lhsT=v_sb[kc][:, h * 65:(h + 1) * 65],
                                rhs=em2[:, a * 512:(a + 1) * 512],
                                start=(kc == 0),
                                stop=(kc == KC - 1),
                            )
                    for a, h in enumerate((h0, h1)):
                        acc = accs[a]
                        # free the PSUM accumulator quickly (copies on ACT,
                        # which has headroom; DVE carries the mask muls);
                        # the rest of the normalization is deferred into the
                        # next head-pair's loop.
                        ut = work.tile(
                            [HD, 512], f32, tag="ut", name=f"ut{h}_{t}", bufs=5
                        )
                        nc.vector.tensor_copy(ut, acc[0:HD, :])
                        s_f = work.tile(
                            [1, 512], f32, tag="s", name=f"s{h}_{t}", bufs=5
                        )
                        nc.scalar.copy(s_f, acc[64:65, :])
                        norm_pending.append(make_norm(hp, t, a, h, ut, s_f))

            while norm_pending:
                norm_pending.popleft()()

            # ---- output projection ----
            for t2 in range(NQ // P):
                for n in range(D // 512):
                    ps_o = pmm.tile([P, 512], f32, tag="mm", name=f"ps_o{t2}_{n}")
                    for c in range(MT):
                        nc.tensor.matmul(
                            ps_o,
                            lhsT=vals_sb[c][:, t2 * P:(t2 + 1) * P],
                            rhs=wo_sb[c][:, n * 512:(n + 1) * 512],
                            start=(c == 0),
                            stop=(c == MT - 1),
                        )
                    ot = work.tile([P, 512], f32, tag="ot", name=f"ot{t2}_{n}", bufs=2)
                    nc.scalar.copy(ot, ps_o)
                    nc.sync.dma_start(
                        out_d[t2 * P:(t2 + 1) * P, n * 512:(n + 1) * 512], ot
                    )

    nc.compile()
    return nc


def _get_program():
    if "nc" not in _CACHE:
        _CACHE["nc"] = _build_program()
    return _CACHE["nc"]


def _per_core_inputs(x, y, mask, W_kv, b_kv, W_q, b_q, W_o):
    """Build the 8 per-core input maps."""
    in_maps = []
    mask_f = mask.astype(np.float32)
    for c in range(8):
        b, g = c // 2, c % 2
        gh = np.arange(g * 8, g * 8 + 8)
        k_cols = (gh[:, None] * 2 * HD + np.arange(HD)[None, :]).ravel()
        v_cols = k_cols + HD
        q_cols = slice(g * 512, (g + 1) * 512)
        in_maps.append({
            "xT": np.ascontiguousarray(x[b].T).astype(BF),
            "yT": np.ascontiguousarray(y[b].T).astype(BF),
            "maskT": np.ascontiguousarray(mask_f[b].T).astype(BF),
            "wk": np.ascontiguousarray(W_kv[:, k_cols]).astype(BF),
            "wv": np.ascontiguousarray(W_kv[:, v_cols]).astype(BF),
            "wq": np.ascontiguousarray(W_q[:, q_cols]).astype(BF),
            "wo": np.ascontiguousarray(W_o[q_cols, :]).astype(BF),
            "bk": b_kv[k_cols].astype(np.float32).reshape(512, 1),
            "bq": b_q[np.arange(g * 512, (g + 1) * 512)]
                  .astype(np.float32).reshape(512, 1),
        })
    return in_maps


def kernel(x, y, mask, W_kv, b_kv, W_q, b_q, W_o, b_o):
    from concourse import bass_utils

    x = np.asarray(x, np.float32)
    y = np.asarray(y, np.float32)
    mask = np.asarray(mask)
    W_kv = np.asarray(W_kv, np.float32)
    b_kv = np.asarray(b_kv, np.float32)
    W_q = np.asarray(W_q, np.float32)
    b_q = np.asarray(b_q, np.float32)
    W_o = np.asarray(W_o, np.float32)
    b_o = np.asarray(b_o, np.float32)

    nc = _get_program()
    in_maps = _per_core_inputs(x, y, mask, W_kv, b_kv, W_q, b_q, W_o)
    res = bass_utils.run_bass_kernel_spmd(nc, in_maps, core_ids=list(range(8)))

    # b_v folds into a constant row: attn rows sum to 1, so each head adds
    # b_v_h @ W_o_h to every output row; b_o adds on top.
    v_cols_all = (np.arange(H)[:, None] * 2 * HD + HD
                  + np.arange(HD)[None, :]).ravel()
    const_row = b_kv[v_cols_all].astype(np.float32) @ W_o + b_o

    out = np.empty((B, NQ, D), np.float32)
    for b in range(B):
        out[b] = (res.results[2 * b]["out"] + res.results[2 * b + 1]["out"]
                  + const_row)
    return out


if __name__ == "__main__":
    import reference

    inputs = {k: np.asarray(v) for k, v in reference.setup_inputs().items()}
    got = kernel(**inputs)
    exp = np.asarray(reference.reference(**inputs))
    err = np.abs(got - exp)
    print("absmax rel err:", err.max() / np.abs(exp).max())
